# revision 1
# baseline (speedup 1.0000x reference)
"""Trainium2 Bass kernel for a 2-layer GATv2 encoder (nn_CG_GNN_Encoder).

Fully self-contained: kernel(**inputs) takes the full-size inputs
(x [20000,512] f32, edge_index [2,320000] int64, weights) and returns the
full [20000, 512] f32 output, distributing work across 8 NeuronCores.

Strategy (graph/data parallel over destination nodes):
  - Nodes are assigned to 8 cores x 20 blocks x 125 dst-nodes/block by a
    greedy balance of in-edge counts; edges (incl. self-loops) grouped by
    owning block and padded to a uniform EBLK.
  - Per layer: each core computes xl/xr for its own node shard (PE matmuls,
    bf16), all-gathers xl across cores, keeps xr local.
  - Edge phase per block: dma_gather xl[src] and xr[dst] rows (bf16),
    t = leaky_relu(xl+xr), per-head logits via sign-range reduces (the
    attention vector's magnitudes are folded into the weights on the host,
    its signs into a column ordering), p = exp(logits + padmask),
    segment-softmax and alpha-weighted aggregation via one-hot matmuls
    accumulated in PSUM, then normalize + bias (+ ELU between layers).
"""

import numpy as np
from ml_dtypes import bfloat16

import concourse.bacc as bacc
import concourse.bass as bass
import concourse.mybir as mybir
import concourse.tile as tile
from concourse.bass_utils import run_bass_kernel_spmd
from concourse.masks import make_identity

F32 = mybir.dt.float32
BF16 = mybir.dt.bfloat16
I16 = mybir.dt.int16
I32 = mybir.dt.int32
AX = mybir.AxisListType
OP = mybir.AluOpType
ACT = mybir.ActivationFunctionType

N = 20000
H = 4
C = 128
IN = 512
HC = H * C
NEG = 0.2
NCORES = 8
NSH = N // NCORES      # 2500
DBLK = 125             # dst nodes per block (also phase-A node-tile rows)
NBLK = NSH // DBLK     # 20
ATT_EPS = 1e-10


# ----------------------------------------------------------------------------
# Host-side preprocessing
# ----------------------------------------------------------------------------

def _wrap16(idx, e_blk):
    out = np.zeros((16, e_blk // 16), idx.dtype)
    pos = np.arange(len(idx))
    out[pos % 16, pos // 16] = idx
    return out


def _preprocess_graph(edge_index):
    src = np.concatenate([edge_index[0], np.arange(N, dtype=np.int64)])
    dst = np.concatenate([edge_index[1], np.arange(N, dtype=np.int64)])
    deg = np.bincount(dst, minlength=N)

    nbins = NCORES * NBLK
    order = np.argsort(-deg, kind="stable")
    import heapq
    bin_load = np.zeros(nbins, np.int64)
    bin_fill = np.zeros(nbins, np.int64)
    assign = np.zeros(N, np.int64)
    heap = [(0, b) for b in range(nbins)]
    heapq.heapify(heap)
    for nid in order:
        while True:
            load, b = heapq.heappop(heap)
            if bin_fill[b] < DBLK:
                break
        assign[nid] = b
        bin_fill[b] += 1
        bin_load[b] = load + deg[nid]
        if bin_fill[b] < DBLK:
            heapq.heappush(heap, (bin_load[b], b))

    perm = np.argsort(assign * N + np.arange(N), kind="stable")
    inv_perm = np.empty(N, np.int64)
    inv_perm[perm] = np.arange(N)

    e_bin = assign[dst]
    e_dst_pos = inv_perm[dst]
    e_src_pos = inv_perm[src]
    max_per_bin = int(np.bincount(e_bin, minlength=nbins).max())
    e_blk = -(-max_per_bin // 128) * 128

    order_e = np.argsort(e_bin, kind="stable")
    eb = e_bin[order_e]
    starts = np.searchsorted(eb, np.arange(nbins))
    ends = np.searchsorted(eb, np.arange(nbins), side="right")

    S = e_blk // 128
    src_idx = np.zeros((NCORES, NBLK, 128, S), np.int32)
    dst_idx = np.zeros((NCORES, NBLK, 128, S), np.int32)
    dst_local = np.zeros((NCORES, NBLK, 128, S), np.float32)
    pad_mask = np.full((NCORES, NBLK, 128, S), -1e30, np.float32)

    for b in range(nbins):
        core, blk = divmod(b, NBLK)
        sel = order_e[starts[b]:ends[b]]
        n = len(sel)
        pos = np.arange(n)
        src_idx[core, blk, pos % 128, pos // 128] = e_src_pos[sel]
        dst_idx[core, blk, pos % 128, pos // 128] = e_dst_pos[sel] % NSH
        dst_local[core, blk, pos % 128, pos // 128] = (e_dst_pos[sel] % DBLK)
        pad_mask[core, blk, pos % 128, pos // 128] = 0.0

    per_core = []
    for core in range(NCORES):
        per_core.append(dict(
            src_idx=src_idx[core].transpose(1, 0, 2).reshape(128, -1).copy(),
            dst_idx=dst_idx[core].transpose(1, 0, 2).reshape(128, -1).copy(),
            dst_local=dst_local[core].transpose(1, 0, 2).reshape(128, -1).copy(),
            pad_mask=pad_mask[core].transpose(1, 0, 2).reshape(128, -1).copy(),
        ))
    return per_core, dict(e_blk=e_blk, perm=perm)


def _prep_weights(inputs):
    """Per layer: fold |att| into Wl/Wr columns, order columns pos-signs-first
    per head, row-permute layer-1 weights by layer-0's column order."""
    out = {}
    npos = []
    col_perms = []
    for l in range(2):
        att = np.asarray(inputs[f"att{l}"], np.float32)          # [H, C]
        cols = []
        np_l = []
        scale = np.zeros(HC, np.float32)
        for h in range(H):
            pos = np.where(att[h] >= 0)[0]
            neg = np.where(att[h] < 0)[0]
            ordh = np.concatenate([pos, neg])
            cols.append(h * C + ordh)
            np_l.append(len(pos))
        cols = np.concatenate(cols)                              # new j -> old col
        scale = np.maximum(np.abs(att.reshape(HC)[cols]), ATT_EPS)
        col_perms.append(cols)
        npos.append(np_l)

        Wl = np.asarray(inputs[f"Wl{l}"], np.float32)
        Wr = np.asarray(inputs[f"Wr{l}"], np.float32)
        if l == 1:
            Wl = Wl[col_perms[0], :]
            Wr = Wr[col_perms[0], :]
        out[f"wl{l}"] = (Wl[:, cols] * scale[None, :]).astype(bfloat16)
        out[f"wr{l}"] = (Wr[:, cols] * scale[None, :]).astype(bfloat16)
        aux = np.stack([
            np.asarray(inputs[f"bl{l}"], np.float32)[cols] * scale,
            np.asarray(inputs[f"br{l}"], np.float32)[cols] * scale,
            1.0 / scale,
            np.asarray(inputs[f"bias{l}"], np.float32)[cols],
        ])
        out[f"aux{l}"] = aux.astype(np.float32)                  # [4, 512]
    return out, npos, col_perms


# ----------------------------------------------------------------------------
# Device kernel builder
# ----------------------------------------------------------------------------

def _build(e_blk, npos, mode="full"):
    S = e_blk // 128
    nc = bacc.Bacc("TRN2", target_bir_lowering=False, debug=False,
                   num_devices=NCORES)

    x_in = nc.dram_tensor("x_shard", [NSH, IN], F32, kind="ExternalInput")
    wl_d = [nc.dram_tensor(f"wl{l}", [IN, HC], BF16, kind="ExternalInput")
            for l in range(2)]
    wr_d = [nc.dram_tensor(f"wr{l}", [IN, HC], BF16, kind="ExternalInput")
            for l in range(2)]
    aux_d = [nc.dram_tensor(f"aux{l}", [4, HC], F32, kind="ExternalInput")
             for l in range(2)]
    srcidx_d = nc.dram_tensor("src_idx", [128, NBLK * e_blk // 128], I32,
                              kind="ExternalInput")
    dstidx_d = nc.dram_tensor("dst_idx", [128, NBLK * e_blk // 128], I32,
                              kind="ExternalInput")
    dl_d = nc.dram_tensor("dst_local", [128, NBLK * S], F32,
                          kind="ExternalInput")
    pm_d = nc.dram_tensor("pad_mask", [128, NBLK * S], F32,
                          kind="ExternalInput")
    out_d = nc.dram_tensor("out", [NSH, HC], F32, kind="ExternalOutput")

    with tile.TileContext(nc) as tc:
        with tc.tile_pool(name="dram", bufs=1, space="DRAM") as dram, \
             tc.tile_pool(name="const", bufs=1) as cp, \
             tc.tile_pool(name="work", bufs=2) as wp, \
             tc.tile_pool(name="gath", bufs=2) as gp, \
             tc.tile_pool(name="wh", bufs=12) as whp, \
             tc.tile_pool(name="psum", bufs=2, space="PSUM") as pp:

            xl_sh = [dram.tile([NSH, HC], BF16, name=f"xl_sh{l}") for l in range(2)]
            xr_sh = [dram.tile([NSH, HC], BF16, name=f"xr_sh{l}") for l in range(2)]
            xl_full = [dram.tile([N, HC], BF16, name=f"xl_full{l}")
                       for l in range(2)]
            xl_loc = [dram.tile([N, HC], BF16, name=f"xl_loc{l}")
                      for l in range(2)]
            h_mid = dram.tile([NSH, HC], F32, name="h_mid")

            # constants
            ident = cp.tile([DBLK, DBLK], BF16, name="ident")
            make_identity(nc, ident[:])
            iota_i16 = cp.tile([128, DBLK], I16, name="iota_i16")
            nc.gpsimd.iota(iota_i16[:], pattern=[[1, DBLK]], base=0,
                           channel_multiplier=0)
            iota_bf = cp.tile([128, DBLK], BF16, name="iota_bf")
            nc.vector.tensor_copy(out=iota_bf[:], in_=iota_i16[:])

            si_t = cp.tile([128, NBLK * S], I32, name="si_t")
            di_t = cp.tile([128, NBLK * S], I32, name="di_t")
            dl_t = cp.tile([128, NBLK * S], F32, name="dl_t")
            pm_t = cp.tile([128, NBLK * S], F32, name="pm_t")
            nc.sync.dma_start(out=si_t[:], in_=srcidx_d[:])
            nc.sync.dma_start(out=di_t[:], in_=dstidx_d[:])
            nc.sync.dma_start(out=dl_t[:], in_=dl_d[:])
            nc.sync.dma_start(out=pm_t[:], in_=pm_d[:])

            for l in range(2):
                # ---- phase A: xl/xr shard matmuls --------------------------
                wl_t = cp.tile([128, 4, HC], BF16, name="wl_t", tag="wl_t")
                wr_t = cp.tile([128, 4, HC], BF16, name="wr_t", tag="wr_t")
                for k in range(4):
                    nc.sync.dma_start(out=wl_t[:, k, :],
                                      in_=wl_d[l][k * 128:(k + 1) * 128, :])
                    nc.sync.dma_start(out=wr_t[:, k, :],
                                      in_=wr_d[l][k * 128:(k + 1) * 128, :])
                aux_b = []
                for r in range(4):
                    row = cp.tile([1, HC], F32, name=f"auxrow{r}", tag=f"auxr{r}")
                    nc.sync.dma_start(out=row[:], in_=aux_d[l][r:r + 1, :])
                    bc = cp.tile([128, HC], F32, name=f"auxb{r}", tag=f"auxb{r}")
                    nc.gpsimd.partition_broadcast(bc[:], row[:])
                    aux_b.append(bc)
                bl_b, br_b, invatt_b, bias_b = aux_b

                src_x = x_in if l == 0 else h_mid
                for t in range(NBLK):
                    x_t = wp.tile([DBLK, IN], BF16, name="x_t", tag="x_t")
                    nc.gpsimd.dma_start(
                        out=x_t[:], in_=src_x[t * DBLK:(t + 1) * DBLK, :])
                    xT = wp.tile([128, 4, DBLK], BF16, name="xT", tag="xT")
                    for k in range(4):
                        ps_tr = pp.tile([128, DBLK], BF16, name="ps_tr",
                                        tag="ps_a")
                        nc.tensor.transpose(out=ps_tr[:],
                                            in_=x_t[:, k * 128:(k + 1) * 128],
                                            identity=ident[:])
                        nc.scalar.copy(out=xT[:, k, :], in_=ps_tr[:])
                    ps_xl = pp.tile([DBLK, HC], F32, name="ps_xl", tag="ps_b")
                    ps_xr = pp.tile([DBLK, HC], F32, name="ps_xr", tag="ps_c")
                    for k in range(4):
                        nc.tensor.matmul(out=ps_xl[:], lhsT=xT[:, k, :],
                                         rhs=wl_t[:, k, :],
                                         start=(k == 0), stop=(k == 3))
                    for k in range(4):
                        nc.tensor.matmul(out=ps_xr[:], lhsT=xT[:, k, :],
                                         rhs=wr_t[:, k, :],
                                         start=(k == 0), stop=(k == 3))
                    xl_o = wp.tile([DBLK, HC], BF16, name="xl_o", tag="xl_o")
                    xr_o = wp.tile([DBLK, HC], BF16, name="xr_o", tag="xr_o")
                    nc.vector.tensor_add(out=xl_o[:], in0=ps_xl[:],
                                         in1=bl_b[:DBLK, :])
                    nc.vector.tensor_add(out=xr_o[:], in0=ps_xr[:],
                                         in1=br_b[:DBLK, :])
                    nc.sync.dma_start(out=xl_sh[l][t * DBLK:(t + 1) * DBLK, :],
                                      in_=xl_o[:])
                    nc.sync.dma_start(out=xr_sh[l][t * DBLK:(t + 1) * DBLK, :],
                                      in_=xr_o[:])

                nc.gpsimd.collective_compute(
                    "AllGather", OP.bypass,
                    replica_groups=[list(range(NCORES))],
                    ins=[xl_sh[l][:]], outs=[xl_full[l][:]],
                )
                nc.sync.dma_start(out=xl_loc[l][:], in_=xl_full[l][:])

                if mode == "phasea":
                    nc.gpsimd.dma_start(out=out_d[0:NSH, :],
                                        in_=xl_full[l][0:NSH, :])
                    break
                if mode in ("ig_blk", "ig_direct"):
                    if mode == "ig_blk":
                        srcten = dram.tile([N, HC], BF16, name="xl_loc")
                        nc.sync.dma_start(out=srcten[:], in_=xl_full[l][:])
                        idxap = si_t[:, 0:S]
                        xg = gp.tile([128, S, HC], BF16, name="xg", tag="xl_e")
                    else:
                        srcten = xl_full[l]
                        idxap = si_t[:, 0:1]
                        xg = gp.tile([128, 1, HC], BF16, name="xg", tag="xl_e")
                    nc.gpsimd.indirect_dma_start(
                        out=xg[:], out_offset=None, in_=srcten[:],
                        in_offset=bass.IndirectOffsetOnAxis(ap=idxap, axis=0))
                    nsl = xg.shape[1]
                    for s in range(nsl):
                        nc.gpsimd.dma_start(out=out_d[s*128:(s+1)*128, :],
                                            in_=xg[:, s, :])
                    break
                # ---- phase B: edge blocks ----------------------------------
                for b in range(NBLK):
                    xl_e = gp.tile([128, S, HC], BF16, name="xl_e", tag="xl_e")
                    xr_e = gp.tile([128, S, HC], BF16, name="xr_e", tag="xr_e")
                    for s in range(S):
                        ic = b * S + s
                        nc.gpsimd.indirect_dma_start(
                            out=xl_e[:, s, :], out_offset=None,
                            in_=xl_loc[l][:],
                            in_offset=bass.IndirectOffsetOnAxis(
                                ap=si_t[:, ic:ic + 1], axis=0))
                        nc.gpsimd.indirect_dma_start(
                            out=xr_e[:, s, :], out_offset=None,
                            in_=xr_sh[l][:],
                            in_offset=bass.IndirectOffsetOnAxis(
                                ap=di_t[:, ic:ic + 1], axis=0))

                    t_blk = gp.tile([128, S, HC], BF16, name="t_blk",
                                    tag="t_blk", bufs=1)
                    nc.vector.tensor_add(out=t_blk[:], in0=xl_e[:],
                                         in1=xr_e[:])
                    # leaky_relu(z) = 0.6*z + 0.4*|z|; the range-sums below
                    # use  sum lrelu = 0.6*(sum z + (2/3) sum |z|)
                    u_blk = gp.tile([128, S, HC], BF16, name="u_blk",
                                    tag="u_blk", bufs=1)
                    nc.scalar.activation(out=u_blk[:], in_=t_blk[:],
                                         func=ACT.Abs)

                    lg_pn = wp.tile([128, S, H, 4], F32, name="lg_pn",
                                    tag="lg_pn")
                    for h in range(H):
                        np_h = npos[l][h]
                        for base, tl in ((0, t_blk), (2, u_blk)):
                            if np_h > 0:
                                nc.vector.tensor_reduce(
                                    out=lg_pn[:, :, h, base],
                                    in_=tl[:, :, h * C:h * C + np_h],
                                    axis=AX.X, op=OP.add)
                            else:
                                nc.vector.memset(lg_pn[:, :, h, base], 0.0)
                            if np_h < C:
                                nc.vector.tensor_reduce(
                                    out=lg_pn[:, :, h, base + 1],
                                    in_=tl[:, :, h * C + np_h:(h + 1) * C],
                                    axis=AX.X, op=OP.add)
                            else:
                                nc.vector.memset(lg_pn[:, :, h, base + 1], 0.0)
                    lg2 = wp.tile([128, S, H], F32, name="lg2", tag="lg2")
                    nc.vector.tensor_tensor(out=lg2[:], in0=lg_pn[:, :, :, 2],
                                            in1=lg_pn[:, :, :, 3],
                                            op=OP.subtract)
                    nc.vector.tensor_scalar(out=lg2[:], in0=lg2[:],
                                            scalar1=2.0 / 3.0, scalar2=None,
                                            op0=OP.mult)
                    lg = wp.tile([128, S, H], F32, name="lg", tag="lg")
                    nc.vector.tensor_tensor(out=lg[:], in0=lg_pn[:, :, :, 0],
                                            in1=lg_pn[:, :, :, 1],
                                            op=OP.subtract)
                    nc.vector.tensor_tensor(out=lg[:], in0=lg[:], in1=lg2[:],
                                            op=OP.add)
                    pm_sl = pm_t[:, b * S:(b + 1) * S]
                    nc.vector.tensor_tensor(
                        out=lg[:], in0=lg[:],
                        in1=pm_sl[:, :, None].to_broadcast([128, S, H]),
                        op=OP.add)
                    p_f32 = wp.tile([128, S, H], F32, name="p_f32",
                                    tag="p_f32")
                    nc.scalar.activation(out=p_f32[:], in_=lg[:], func=ACT.Exp,
                                         scale=0.6)
                    p_blk = wp.tile([128, S, H], BF16, name="p_blk",
                                    tag="p_blk")
                    nc.vector.tensor_copy(out=p_blk[:], in_=p_f32[:])

                    ps_o = pp.tile([DBLK, HC], F32, name="ps_o", tag="ps_b")
                    ps_s = pp.tile([DBLK, H], F32, name="ps_s", tag="ps_a")
                    for s in range(S):
                        dl_col = dl_t[:, b * S + s:b * S + s + 1]
                        oh = whp.tile([128, DBLK], BF16, name="oh", tag="oh")
                        nc.vector.tensor_scalar(
                            out=oh[:], in0=iota_bf[:], scalar1=dl_col,
                            scalar2=None, op0=OP.is_equal)
                        nc.tensor.matmul(out=ps_s[:], lhsT=oh[:],
                                         rhs=p_blk[:, s, :],
                                         start=(s == 0), stop=(s == S - 1))
                        for h in range(H):
                            wh = whp.tile([128, DBLK], BF16, name="wh",
                                          tag="wh")
                            nc.vector.tensor_scalar(
                                out=wh[:], in0=iota_bf[:], scalar1=dl_col,
                                scalar2=p_f32[:, s, h:h + 1],
                                op0=OP.is_equal, op1=OP.mult)
                            nc.tensor.matmul(
                                out=ps_o[:, h * C:(h + 1) * C], lhsT=wh[:],
                                rhs=xl_e[:, s, h * C:(h + 1) * C],
                                start=(s == 0 and h == 0),
                                stop=(s == S - 1 and h == H - 1))

                    s_sb = wp.tile([DBLK, H], F32, name="s_sb", tag="s_sb")
                    nc.scalar.copy(out=s_sb[:], in_=ps_s[:])
                    rinv = wp.tile([DBLK, H], F32, name="rinv", tag="rinv")
                    nc.vector.reciprocal(out=rinv[:], in_=s_sb[:])
                    o_sb = wp.tile([DBLK, HC], F32, name="o_sb", tag="o_sb")
                    for h in range(H):
                        nc.vector.tensor_scalar(
                            out=o_sb[:, h * C:(h + 1) * C],
                            in0=ps_o[:, h * C:(h + 1) * C],
                            scalar1=rinv[:, h:h + 1], scalar2=None,
                            op0=OP.mult)
                    nc.vector.tensor_mul(out=o_sb[:], in0=o_sb[:],
                                         in1=invatt_b[:DBLK, :])
                    nc.vector.tensor_add(out=o_sb[:], in0=o_sb[:],
                                         in1=bias_b[:DBLK, :])
                    rows = slice(b * DBLK, (b + 1) * DBLK)
                    if l == 0:
                        r_t = wp.tile([DBLK, HC], F32, name="r_t", tag="r_t")
                        nc.scalar.activation(out=r_t[:], in_=o_sb[:],
                                             func=ACT.Relu)
                        e_t = wp.tile([DBLK, HC], F32, name="e_t", tag="e_t")
                        nc.scalar.activation(out=e_t[:], in_=o_sb[:],
                                             func=ACT.Exp)
                        m_t = wp.tile([DBLK, HC], F32, name="m_t", tag="m_t")
                        nc.vector.tensor_scalar(
                            out=m_t[:], in0=e_t[:], scalar1=-1.0, scalar2=0.0,
                            op0=OP.add, op1=OP.min)
                        h_sb = wp.tile([DBLK, HC], F32, name="h_sb", tag="h_sb")
                        nc.vector.tensor_add(out=h_sb[:], in0=r_t[:],
                                             in1=m_t[:])
                        nc.sync.dma_start(out=h_mid[rows, :], in_=h_sb[:])
                    else:
                        nc.sync.dma_start(out=out_d[rows, :], in_=o_sb[:])

    nc.compile()
    return nc


_CACHE = {}


def _get_nc(e_blk, npos_key):
    key = (e_blk, npos_key)
    if key not in _CACHE:
        _CACHE[key] = _build(e_blk, [list(npos_key[0]), list(npos_key[1])])
    return _CACHE[key]


def kernel(**inputs):
    per_core, meta = _preprocess_graph(np.asarray(inputs["edge_index"]))
    wprep, npos, col_perms = _prep_weights(inputs)
    e_blk = meta["e_blk"]
    perm = meta["perm"]

    nc = _get_nc(e_blk, (tuple(npos[0]), tuple(npos[1])))

    x = np.asarray(inputs["x"], np.float32)
    x_perm = x[perm]
    in_maps = []
    for core in range(NCORES):
        m = dict(
            x_shard=np.ascontiguousarray(x_perm[core * NSH:(core + 1) * NSH]),
            src_idx=per_core[core]["src_idx"],
            dst_idx=per_core[core]["dst_idx"],
            dst_local=per_core[core]["dst_local"],
            pad_mask=per_core[core]["pad_mask"],
        )
        for l in range(2):
            m[f"wl{l}"] = wprep[f"wl{l}"]
            m[f"wr{l}"] = wprep[f"wr{l}"]
            m[f"aux{l}"] = wprep[f"aux{l}"]
        in_maps.append(m)

    trace = bool(inputs.pop("_trace", False))
    res = run_bass_kernel_spmd(nc, in_maps, core_ids=list(range(NCORES)),
                               trace=trace)
    out_rows = np.concatenate([res.results[c]["out"] for c in range(NCORES)],
                              axis=0)
    out = np.zeros((N, HC), np.float32)
    # device rows are in perm order; device cols j correspond to true col
    # col_perms[1][j]
    tmp = np.zeros((N, HC), np.float32)
    tmp[perm] = out_rows
    out[:, col_perms[1]] = tmp
    if trace:
        kernel._last_result = res
    return out



# revision 2
# speedup vs baseline: 1.0167x; 1.0167x over previous
"""Trainium2 Bass kernel for a 2-layer GATv2 encoder (nn_CG_GNN_Encoder).

Fully self-contained: kernel(**inputs) takes the full-size inputs
(x [20000,512] f32, edge_index [2,320000] int64, weights) and returns the
full [20000, 512] f32 output, distributing work across 8 NeuronCores.

Strategy (graph/data parallel over destination nodes):
  - Nodes are assigned to 8 cores x 20 blocks x 125 dst-nodes/block by a
    greedy balance of in-edge counts; edges (incl. self-loops) grouped by
    owning block and padded to a uniform EBLK.
  - Per layer: each core computes xl/xr for its own node shard (PE matmuls,
    bf16), all-gathers xl across cores, keeps xr local.
  - Edge phase per block: dma_gather xl[src] and xr[dst] rows (bf16),
    t = leaky_relu(xl+xr), per-head logits via sign-range reduces (the
    attention vector's magnitudes are folded into the weights on the host,
    its signs into a column ordering), p = exp(logits + padmask),
    segment-softmax and alpha-weighted aggregation via one-hot matmuls
    accumulated in PSUM, then normalize + bias (+ ELU between layers).
"""

import numpy as np
from ml_dtypes import bfloat16

import concourse.bacc as bacc
import concourse.bass as bass
import concourse.mybir as mybir
import concourse.tile as tile
from concourse.bass_utils import run_bass_kernel_spmd
from concourse.masks import make_identity

F32 = mybir.dt.float32
BF16 = mybir.dt.bfloat16
I16 = mybir.dt.int16
I32 = mybir.dt.int32
AX = mybir.AxisListType
OP = mybir.AluOpType
ACT = mybir.ActivationFunctionType

N = 20000
H = 4
C = 128
IN = 512
HC = H * C
NEG = 0.2
NCORES = 8
NSH = N // NCORES      # 2500
DBLK = 125             # dst nodes per block (also phase-A node-tile rows)
NBLK = NSH // DBLK     # 20
ATT_EPS = 1e-10
ZC = HC + 4            # 516: values + 4 z-part cols


# ----------------------------------------------------------------------------
# Host-side preprocessing
# ----------------------------------------------------------------------------

def _wrap16(idx, e_blk):
    out = np.zeros((16, e_blk // 16), idx.dtype)
    pos = np.arange(len(idx))
    out[pos % 16, pos // 16] = idx
    return out


def _preprocess_graph(edge_index):
    src = np.concatenate([edge_index[0], np.arange(N, dtype=np.int64)])
    dst = np.concatenate([edge_index[1], np.arange(N, dtype=np.int64)])
    deg = np.bincount(dst, minlength=N)

    nbins = NCORES * NBLK
    order = np.argsort(-deg, kind="stable")
    import heapq
    bin_load = np.zeros(nbins, np.int64)
    bin_fill = np.zeros(nbins, np.int64)
    assign = np.zeros(N, np.int64)
    heap = [(0, b) for b in range(nbins)]
    heapq.heapify(heap)
    for nid in order:
        while True:
            load, b = heapq.heappop(heap)
            if bin_fill[b] < DBLK:
                break
        assign[nid] = b
        bin_fill[b] += 1
        bin_load[b] = load + deg[nid]
        if bin_fill[b] < DBLK:
            heapq.heappush(heap, (bin_load[b], b))

    perm = np.argsort(assign * N + np.arange(N), kind="stable")
    inv_perm = np.empty(N, np.int64)
    inv_perm[perm] = np.arange(N)

    e_bin = assign[dst]
    e_dst_pos = inv_perm[dst]
    e_src_pos = inv_perm[src]
    max_per_bin = int(np.bincount(e_bin, minlength=nbins).max())
    e_blk = -(-max_per_bin // 128) * 128

    order_e = np.argsort(e_bin, kind="stable")
    eb = e_bin[order_e]
    starts = np.searchsorted(eb, np.arange(nbins))
    ends = np.searchsorted(eb, np.arange(nbins), side="right")

    S = e_blk // 128
    src_idx = np.zeros((NCORES, NBLK, 128, S), np.int32)
    dst_idx = np.zeros((NCORES, NBLK, 128, S), np.int32)
    dst_local = np.zeros((NCORES, NBLK, 128, S), np.float32)
    pad_mask = np.full((NCORES, NBLK, 128, S), -1e30, np.float32)

    for b in range(nbins):
        core, blk = divmod(b, NBLK)
        sel = order_e[starts[b]:ends[b]]
        n = len(sel)
        pos = np.arange(n)
        src_idx[core, blk, pos % 128, pos // 128] = e_src_pos[sel]
        dst_idx[core, blk, pos % 128, pos // 128] = e_dst_pos[sel] % NSH
        dst_local[core, blk, pos % 128, pos // 128] = (e_dst_pos[sel] % DBLK)
        pad_mask[core, blk, pos % 128, pos // 128] = 0.0

    per_core = []
    for core in range(NCORES):
        per_core.append(dict(
            src_idx=src_idx[core].transpose(1, 0, 2).reshape(128, -1).copy(),
            dst_idx=dst_idx[core].transpose(1, 0, 2).reshape(128, -1).copy(),
            dst_local=dst_local[core].transpose(1, 0, 2).reshape(128, -1).copy(),
            pad_mask=pad_mask[core].transpose(1, 0, 2).reshape(128, -1).copy(),
        ))
    return per_core, dict(e_blk=e_blk, perm=perm)


def _prep_weights(inputs):
    """Fold |att| into W cols (pos-first per head); append Wz = sum of
    +/- signed cols per head so phase A emits the src z-part directly."""
    out = {}
    npos = []
    col_perms = []
    for l in range(2):
        att = np.asarray(inputs[f"att{l}"], np.float32)          # [H, C]
        cols = []
        np_l = []
        for h in range(H):
            pos = np.where(att[h] >= 0)[0]
            neg = np.where(att[h] < 0)[0]
            cols.append(h * C + np.concatenate([pos, neg]))
            np_l.append(len(pos))
        cols = np.concatenate(cols)
        scale = np.maximum(np.abs(att.reshape(HC)[cols]), ATT_EPS)
        col_perms.append(cols)
        npos.append(np_l)
        sgn = np.ones(HC, np.float32)
        for h in range(H):
            sgn[h * C + np_l[h]:(h + 1) * C] = -1.0

        Wl = np.asarray(inputs[f"Wl{l}"], np.float32)
        Wr = np.asarray(inputs[f"Wr{l}"], np.float32)
        if l == 1:
            Wl = Wl[col_perms[0], :]
            Wr = Wr[col_perms[0], :]
        Wlt = Wl[:, cols] * scale[None, :]
        Wrt = Wr[:, cols] * scale[None, :]
        blt = np.asarray(inputs[f"bl{l}"], np.float32)[cols] * scale
        brt = np.asarray(inputs[f"br{l}"], np.float32)[cols] * scale
        Wz = np.stack([(Wlt[:, h * C:(h + 1) * C]
                        * sgn[None, h * C:(h + 1) * C]).sum(1)
                       for h in range(H)], axis=1)
        bz = np.stack([(blt[h * C:(h + 1) * C]
                        * sgn[h * C:(h + 1) * C]).sum()
                       for h in range(H)])
        out[f"wl{l}"] = np.concatenate([Wlt, Wz], 1).astype(bfloat16)
        out[f"wr{l}"] = Wrt.astype(bfloat16)
        aux = np.zeros((4, ZC), np.float32)
        aux[0, :HC] = blt
        aux[0, HC:] = bz
        aux[1, :HC] = brt
        aux[2, :HC] = 1.0 / scale
        aux[3, :HC] = np.asarray(inputs[f"bias{l}"], np.float32)[cols]
        out[f"aux{l}"] = aux
    return out, npos, col_perms


# ----------------------------------------------------------------------------
# Device kernel builder
# ----------------------------------------------------------------------------

def _build(e_blk, npos, mode="full"):
    S = e_blk // 128
    nc = bacc.Bacc("TRN2", target_bir_lowering=False, debug=False,
                   num_devices=NCORES)

    x_in = nc.dram_tensor("x_shard", [NSH, IN], F32, kind="ExternalInput")
    wl_d = [nc.dram_tensor(f"wl{l}", [IN, ZC], BF16, kind="ExternalInput")
            for l in range(2)]
    wr_d = [nc.dram_tensor(f"wr{l}", [IN, HC], BF16, kind="ExternalInput")
            for l in range(2)]
    aux_d = [nc.dram_tensor(f"aux{l}", [4, ZC], F32, kind="ExternalInput")
             for l in range(2)]
    srcidx_d = nc.dram_tensor("src_idx", [128, NBLK * e_blk // 128], I32,
                              kind="ExternalInput")
    dstidx_d = nc.dram_tensor("dst_idx", [128, NBLK * e_blk // 128], I32,
                              kind="ExternalInput")
    dl_d = nc.dram_tensor("dst_local", [128, NBLK * S], F32,
                          kind="ExternalInput")
    pm_d = nc.dram_tensor("pad_mask", [128, NBLK * S], F32,
                          kind="ExternalInput")
    out_d = nc.dram_tensor("out", [NSH, HC], F32, kind="ExternalOutput")

    with tile.TileContext(nc) as tc:
        with tc.tile_pool(name="dram", bufs=1, space="DRAM") as dram, \
             tc.tile_pool(name="const", bufs=1) as cp, \
             tc.tile_pool(name="work", bufs=2) as wp, \
             tc.tile_pool(name="gath", bufs=2) as gp, \
             tc.tile_pool(name="wh", bufs=12) as whp, \
             tc.tile_pool(name="psum", bufs=2, space="PSUM") as pp:

            xl_sh = [dram.tile([NSH, ZC], BF16, name=f"xl_sh{l}") for l in range(2)]
            xr_sh = [dram.tile([NSH, HC], BF16, name=f"xr_sh{l}") for l in range(2)]
            xl_full = [dram.tile([N, ZC], BF16, name=f"xl_full{l}")
                       for l in range(2)]
            xl_loc = [dram.tile([N, ZC], BF16, name=f"xl_loc{l}")
                      for l in range(2)]
            h_mid = dram.tile([NSH, HC], F32, name="h_mid")

            # constants
            ident = cp.tile([DBLK, DBLK], BF16, name="ident")
            make_identity(nc, ident[:])
            iota_i16 = cp.tile([128, DBLK], I16, name="iota_i16")
            nc.gpsimd.iota(iota_i16[:], pattern=[[1, DBLK]], base=0,
                           channel_multiplier=0)
            iota_bf = cp.tile([128, DBLK], BF16, name="iota_bf")
            nc.vector.tensor_copy(out=iota_bf[:], in_=iota_i16[:])

            si_t = cp.tile([128, NBLK * S], I32, name="si_t")
            di_t = cp.tile([128, NBLK * S], I32, name="di_t")
            dl_t = cp.tile([128, NBLK * S], F32, name="dl_t")
            pm_t = cp.tile([128, NBLK * S], F32, name="pm_t")
            nc.sync.dma_start(out=si_t[:], in_=srcidx_d[:])
            nc.sync.dma_start(out=di_t[:], in_=dstidx_d[:])
            nc.sync.dma_start(out=dl_t[:], in_=dl_d[:])
            nc.sync.dma_start(out=pm_t[:], in_=pm_d[:])

            for l in range(2):
                # ---- phase A: xl/xr shard matmuls --------------------------
                wl_t = cp.tile([128, 4, ZC], BF16, name="wl_t", tag="wl_t")
                wr_t = cp.tile([128, 4, HC], BF16, name="wr_t", tag="wr_t")
                for k in range(4):
                    nc.sync.dma_start(out=wl_t[:, k, :],
                                      in_=wl_d[l][k * 128:(k + 1) * 128, :])
                    nc.sync.dma_start(out=wr_t[:, k, :],
                                      in_=wr_d[l][k * 128:(k + 1) * 128, :])
                aux_b = []
                for r in range(4):
                    row = cp.tile([1, ZC], F32, name=f"auxrow{r}", tag=f"auxr{r}")
                    nc.sync.dma_start(out=row[:], in_=aux_d[l][r:r + 1, :])
                    bc = cp.tile([128, ZC], F32, name=f"auxb{r}", tag=f"auxb{r}")
                    nc.gpsimd.partition_broadcast(bc[:], row[:])
                    aux_b.append(bc)
                bl_b, br_b, invatt_b, bias_b = aux_b

                src_x = x_in if l == 0 else h_mid
                for t in range(NBLK):
                    x_t = wp.tile([DBLK, IN], BF16, name="x_t", tag="x_t")
                    nc.gpsimd.dma_start(
                        out=x_t[:], in_=src_x[t * DBLK:(t + 1) * DBLK, :])
                    xT = wp.tile([128, 4, DBLK], BF16, name="xT", tag="xT")
                    for k in range(4):
                        ps_tr = pp.tile([128, DBLK], BF16, name="ps_tr",
                                        tag="ps_a")
                        nc.tensor.transpose(out=ps_tr[:],
                                            in_=x_t[:, k * 128:(k + 1) * 128],
                                            identity=ident[:])
                        nc.scalar.copy(out=xT[:, k, :], in_=ps_tr[:])
                    ps_xl = pp.tile([DBLK, HC], F32, name="ps_xl", tag="ps_b")
                    ps_xr = pp.tile([DBLK, HC], F32, name="ps_xr", tag="ps_c")
                    ps_z = pp.tile([DBLK, 4], F32, name="ps_z", tag="ps_z")
                    for k in range(4):
                        nc.tensor.matmul(out=ps_xl[:], lhsT=xT[:, k, :],
                                         rhs=wl_t[:, k, 0:HC],
                                         start=(k == 0), stop=(k == 3))
                    for k in range(4):
                        nc.tensor.matmul(out=ps_z[:], lhsT=xT[:, k, :],
                                         rhs=wl_t[:, k, HC:ZC],
                                         start=(k == 0), stop=(k == 3))
                    for k in range(4):
                        nc.tensor.matmul(out=ps_xr[:], lhsT=xT[:, k, :],
                                         rhs=wr_t[:, k, :],
                                         start=(k == 0), stop=(k == 3))
                    xl_o = wp.tile([DBLK, ZC], BF16, name="xl_o", tag="xl_o")
                    xr_o = wp.tile([DBLK, HC], BF16, name="xr_o", tag="xr_o")
                    nc.vector.tensor_add(out=xl_o[:, 0:HC], in0=ps_xl[:],
                                         in1=bl_b[:DBLK, 0:HC])
                    nc.vector.tensor_add(out=xl_o[:, HC:ZC], in0=ps_z[:],
                                         in1=bl_b[:DBLK, HC:ZC])
                    nc.vector.tensor_add(out=xr_o[:], in0=ps_xr[:],
                                         in1=br_b[:DBLK, 0:HC])
                    nc.sync.dma_start(out=xl_sh[l][t * DBLK:(t + 1) * DBLK, :],
                                      in_=xl_o[:])
                    nc.sync.dma_start(out=xr_sh[l][t * DBLK:(t + 1) * DBLK, :],
                                      in_=xr_o[:])

                nc.gpsimd.collective_compute(
                    "AllGather", OP.bypass,
                    replica_groups=[list(range(NCORES))],
                    ins=[xl_sh[l][:]], outs=[xl_full[l][:]],
                )
                nc.sync.dma_start(out=xl_loc[l][:], in_=xl_full[l][:])

                if mode == "phasea":
                    nc.gpsimd.dma_start(out=out_d[0:NSH, :],
                                        in_=xl_full[l][0:NSH, :])
                    break
                if mode in ("ig_blk", "ig_direct"):
                    if mode == "ig_blk":
                        srcten = dram.tile([N, HC], BF16, name="xl_loc")
                        nc.sync.dma_start(out=srcten[:], in_=xl_full[l][:])
                        idxap = si_t[:, 0:S]
                        xg = gp.tile([128, S, HC], BF16, name="xg", tag="xl_e")
                    else:
                        srcten = xl_full[l]
                        idxap = si_t[:, 0:1]
                        xg = gp.tile([128, 1, HC], BF16, name="xg", tag="xl_e")
                    nc.gpsimd.indirect_dma_start(
                        out=xg[:], out_offset=None, in_=srcten[:],
                        in_offset=bass.IndirectOffsetOnAxis(ap=idxap, axis=0))
                    nsl = xg.shape[1]
                    for s in range(nsl):
                        nc.gpsimd.dma_start(out=out_d[s*128:(s+1)*128, :],
                                            in_=xg[:, s, :])
                    break
                # ---- phase B: edge blocks ----------------------------------
                for b in range(NBLK):
                    xl_e = gp.tile([128, S, ZC], BF16, name="xl_e", tag="xl_e")
                    xr_e = gp.tile([128, S, HC], BF16, name="xr_e", tag="xr_e")
                    for s in range(S):
                        ic = b * S + s
                        nc.gpsimd.indirect_dma_start(
                            out=xl_e[:, s, :], out_offset=None,
                            in_=xl_loc[l][:],
                            in_offset=bass.IndirectOffsetOnAxis(
                                ap=si_t[:, ic:ic + 1], axis=0))
                        nc.gpsimd.indirect_dma_start(
                            out=xr_e[:, s, :], out_offset=None,
                            in_=xr_sh[l][:],
                            in_offset=bass.IndirectOffsetOnAxis(
                                ap=di_t[:, ic:ic + 1], axis=0))

                    t_blk = gp.tile([128, S, HC], BF16, name="t_blk",
                                    tag="t_blk", bufs=1)
                    nc.vector.tensor_add(out=t_blk[:],
                                         in0=xl_e[:, :, 0:HC],
                                         in1=xr_e[:])

                    # |t| reduces over pos/neg ranges per head (abs fused)
                    red = wp.tile([128, S, H, 2], F32, name="red", tag="red")
                    for h in range(H):
                        nph = npos[l][h]
                        if nph > 0:
                            nc.vector.tensor_reduce(
                                out=red[:, :, h, 0],
                                in_=t_blk[:, :, h * C:h * C + nph],
                                axis=AX.X, op=OP.add,
                                apply_absolute_value=True)
                        else:
                            nc.vector.memset(red[:, :, h, 0], 0.0)
                        if nph < C:
                            nc.vector.tensor_reduce(
                                out=red[:, :, h, 1],
                                in_=t_blk[:, :, h * C + nph:(h + 1) * C],
                                axis=AX.X, op=OP.add,
                                apply_absolute_value=True)
                        else:
                            nc.vector.memset(red[:, :, h, 1], 0.0)

                    # lg' = zsl + (2/3)(u+ - u-) + pad;  p = exp(0.6 lg')
                    du = wp.tile([128, S, H], F32, name="du", tag="du")
                    nc.vector.tensor_tensor(out=du[:], in0=red[:, :, :, 0],
                                            in1=red[:, :, :, 1],
                                            op=OP.subtract)
                    lg = wp.tile([128, S, H], F32, name="lg", tag="lg")
                    nc.vector.scalar_tensor_tensor(
                        out=lg[:], in0=du[:], scalar=2.0 / 3.0,
                        in1=xl_e[:, :, HC:ZC], op0=OP.mult, op1=OP.add)
                    pm_sl = pm_t[:, b * S:(b + 1) * S]
                    nc.vector.tensor_tensor(
                        out=lg[:], in0=lg[:],
                        in1=pm_sl[:, :, None].to_broadcast([128, S, H]),
                        op=OP.add)
                    p_f32 = wp.tile([128, S, H], F32, name="p_f32",
                                    tag="p_f32")
                    nc.scalar.activation(out=p_f32[:], in_=lg[:], func=ACT.Exp,
                                         scale=0.6)

                    # xl_e <- xl*p per head in place; p into the z cols
                    for s2 in range(S):
                        for h in range(H):
                            nc.vector.tensor_scalar(
                                out=xl_e[:, s2, h * C:(h + 1) * C],
                                in0=xl_e[:, s2, h * C:(h + 1) * C],
                                scalar1=p_f32[:, s2, h:h + 1], scalar2=None,
                                op0=OP.mult)
                    nc.scalar.copy(out=xl_e[:, :, HC:ZC], in_=p_f32[:])

                    ps_o = pp.tile([DBLK, HC], F32, name="ps_o", tag="ps_b")
                    ps_s = pp.tile([DBLK, H], F32, name="ps_s", tag="ps_a")
                    for s2 in range(S):
                        dl_col = dl_t[:, b * S + s2:b * S + s2 + 1]
                        oh = whp.tile([128, DBLK], BF16, name="oh", tag="oh")
                        nc.vector.tensor_scalar(
                            out=oh[:], in0=iota_bf[:], scalar1=dl_col,
                            scalar2=None, op0=OP.is_equal)
                        nc.tensor.matmul(out=ps_o[:], lhsT=oh[:],
                                         rhs=xl_e[:, s2, 0:HC],
                                         start=(s2 == 0), stop=(s2 == S - 1))
                        nc.tensor.matmul(out=ps_s[:], lhsT=oh[:],
                                         rhs=xl_e[:, s2, HC:ZC],
                                         start=(s2 == 0), stop=(s2 == S - 1))

                    s_sb = wp.tile([DBLK, H], F32, name="s_sb", tag="s_sb")
                    nc.scalar.copy(out=s_sb[:], in_=ps_s[:])
                    rinv = wp.tile([DBLK, H], F32, name="rinv", tag="rinv")
                    nc.vector.reciprocal(out=rinv[:], in_=s_sb[:])
                    o_sb = wp.tile([DBLK, HC], F32, name="o_sb", tag="o_sb")
                    for h in range(H):
                        nc.vector.tensor_scalar(
                            out=o_sb[:, h * C:(h + 1) * C],
                            in0=ps_o[:, h * C:(h + 1) * C],
                            scalar1=rinv[:, h:h + 1], scalar2=None,
                            op0=OP.mult)
                    nc.vector.tensor_mul(out=o_sb[:], in0=o_sb[:],
                                         in1=invatt_b[:DBLK, 0:HC])
                    nc.vector.tensor_add(out=o_sb[:], in0=o_sb[:],
                                         in1=bias_b[:DBLK, 0:HC])
                    rows = slice(b * DBLK, (b + 1) * DBLK)
                    if l == 0:
                        r_t = wp.tile([DBLK, HC], F32, name="r_t", tag="r_t")
                        nc.scalar.activation(out=r_t[:], in_=o_sb[:],
                                             func=ACT.Relu)
                        e_t = wp.tile([DBLK, HC], F32, name="e_t", tag="e_t")
                        nc.scalar.activation(out=e_t[:], in_=o_sb[:],
                                             func=ACT.Exp)
                        m_t = wp.tile([DBLK, HC], F32, name="m_t", tag="m_t")
                        nc.vector.tensor_scalar(
                            out=m_t[:], in0=e_t[:], scalar1=-1.0, scalar2=0.0,
                            op0=OP.add, op1=OP.min)
                        h_sb = wp.tile([DBLK, HC], F32, name="h_sb", tag="h_sb")
                        nc.vector.tensor_add(out=h_sb[:], in0=r_t[:],
                                             in1=m_t[:])
                        nc.sync.dma_start(out=h_mid[rows, :], in_=h_sb[:])
                    else:
                        nc.sync.dma_start(out=out_d[rows, :], in_=o_sb[:])

    nc.compile()
    return nc


_CACHE = {}


def _get_nc(e_blk, npos_key):
    key = (e_blk, npos_key)
    if key not in _CACHE:
        _CACHE[key] = _build(e_blk, [list(npos_key[0]), list(npos_key[1])])
    return _CACHE[key]


def kernel(**inputs):
    per_core, meta = _preprocess_graph(np.asarray(inputs["edge_index"]))
    wprep, npos, col_perms = _prep_weights(inputs)
    e_blk = meta["e_blk"]
    perm = meta["perm"]

    nc = _get_nc(e_blk, (tuple(npos[0]), tuple(npos[1])))

    x = np.asarray(inputs["x"], np.float32)
    x_perm = x[perm]
    in_maps = []
    for core in range(NCORES):
        m = dict(
            x_shard=np.ascontiguousarray(x_perm[core * NSH:(core + 1) * NSH]),
            src_idx=per_core[core]["src_idx"],
            dst_idx=per_core[core]["dst_idx"],
            dst_local=per_core[core]["dst_local"],
            pad_mask=per_core[core]["pad_mask"],
        )
        for l in range(2):
            m[f"wl{l}"] = wprep[f"wl{l}"]
            m[f"wr{l}"] = wprep[f"wr{l}"]
            m[f"aux{l}"] = wprep[f"aux{l}"]
        in_maps.append(m)

    trace = bool(inputs.pop("_trace", False))
    res = run_bass_kernel_spmd(nc, in_maps, core_ids=list(range(NCORES)),
                               trace=trace)
    out_rows = np.concatenate([res.results[c]["out"] for c in range(NCORES)],
                              axis=0)
    out = np.zeros((N, HC), np.float32)
    # device rows are in perm order; device cols j correspond to true col
    # col_perms[1][j]
    tmp = np.zeros((N, HC), np.float32)
    tmp[perm] = out_rows
    out[:, col_perms[1]] = tmp
    if trace:
        kernel._last_result = res
    return out

